# revision 22
# baseline (speedup 1.0000x reference)
"""BasicMoEBlock kernel for Trainium2 (Bass/Tile), data-parallel over batch on 8 cores.

Computation per sample (matches the reference):
    rw1 = avgpool_experts(sigmoid(mean_hw(x) @ r1_W.T + r1_b))
    out = relu(bn1(conv3x3(x, rw1 @ e1_w)))
    rw2 = avgpool_experts(sigmoid(mean_hw(out) @ r2_W.T + r2_b))
    out = relu(bn2(conv3x3(out, rw2 @ e2_w)) + x)

Mapping:
  - conv3x3 = 18 accumulating PE matmuls (2 ci-chunks x 9 shifts) over a
    zero-padded 34x34 image held in SBUF (bf16), fp32 PSUM accumulation.
  - per-sample expert-weight combination on VectorE (tensor_scalar +
    fused MACs, bf16), with one multiply per piece offloaded to ScalarE.
  - expert-avg + all-partition broadcast of routing weights in a single
    constant-mask matmul per band pair (host pre-permutes experts).
  - PE warmup matmuls during the initial DMA window keep HAM at 2.4 GHz.
  - last conv runs per-(co,hh) PSUM groups so its epilogue overlaps the
    final matmul groups instead of serializing after them.
"""

import numpy as np
import ml_dtypes

import concourse.bass as bass
import concourse.tile as tile
from concourse import mybir

F32 = mybir.dt.float32
BF16 = mybir.dt.bfloat16
BF16_NP = ml_dtypes.bfloat16

N_CORES = 8
B_LOC = 4          # samples per core
P = 128            # partitions
CI2 = 2            # input channel chunks (256 = 2*128)
CO2 = 2            # output channel chunks
C = 256
HW = 1024          # 32*32
PADW = 34
PADHW = PADW * PADW
E = 4              # experts
NSH = 9            # 3x3 shifts
QF = NSH * P       # 1152 weight cols per (ci, co) quarter
EPS = 1e-5
N_WARM = 12        # PE warmup matmuls (N=512) to trip the HAM clock gate
AF = mybir.ActivationFunctionType
OP = mybir.AluOpType


# ---------------------------------------------------------------- kernel build

def _declare_io(nc):
    d = {}

    def din(name, shape, dtype):
        d[name] = nc.dram_tensor(name, shape, dtype, kind="ExternalInput").ap()

    # x is pre-cast to bf16 on the host: the conv consumes bf16 anyway, and
    # halving the x bytes shortens the startup-critical DMA stream
    din("x", [B_LOC, C, HW], BF16)
    # experts host-permuted to slot order [0, 2, 1, 3] = (band i, chunk h)
    din("ew1", [P, E, CI2, CO2, QF], BF16)
    din("ew2", [P, E, CI2, CO2, QF], BF16)
    din("rwt", [P, 2, CI2, C], BF16)    # r{1,2}_W.T, [cin_in, layer, cin_chunk, interm]
    # fp32 blob: rb1[2] rb2[2] inv1[2] shift1[2] inv2[2] shift2[2]
    din("fblob", [P, 12], F32)
    d["out"] = nc.dram_tensor("out", [B_LOC, C, HW], F32, kind="ExternalOutput").ap()
    return d


def _emit(tc, d):
    nc = tc.nc

    with (
        tc.tile_pool(name="const", bufs=1) as const,
        tc.tile_pool(name="wcombp", bufs=3) as wcombp,
        tc.tile_pool(name="xin", bufs=4) as xin,
        tc.tile_pool(name="resp", bufs=3) as resp,
        tc.tile_pool(name="rsb", bufs=4) as rsb,
        tc.tile_pool(name="rps", bufs=2, space="PSUM") as rps,
        tc.tile_pool(name="cps", bufs=2, space="PSUM") as cps,
    ):
        # ---- persistent state
        ew_sb = [const.tile([P, E, CI2, CO2, QF], BF16, tag=f"ew{l}", name=f"ew{l}") for l in (0, 1)]
        rwt_all = const.tile([P, 2, CI2, C], BF16, tag="rwtall")
        fblob = const.tile([P, 12], F32, tag="fblob")
        rwt_sb = [rwt_all[:, l] for l in (0, 1)]
        pool_bf = [const.tile([P, B_LOC, CI2], BF16, tag=f"poolbf{l}", name=f"poolbf{l}") for l in (0, 1)]
        rb_sb = [fblob[:, 0:2], fblob[:, 2:4]]
        inv_sb = [fblob[:, 4:6], fblob[:, 8:10]]
        shift_sb = [fblob[:, 6:8], fblob[:, 10:12]]
        junk = const.tile([P, 512], BF16, tag="junk")
        # mask[:, i, :]: lhsT that avgs expert band i (parts 64i..64i+63) of a
        # sigmoid chunk and broadcasts it to all 128 output partitions
        mask = const.tile([P, 2, P], BF16, tag="mask")
        xpad = const.tile([P, B_LOC, CI2, PADHW], BF16, tag="xpad")
        o1pad = const.tile([P, B_LOC, CI2, PADHW], BF16, tag="o1pad")
        pool_acc = [const.tile([P, B_LOC, CI2], F32, tag=f"pool{l}", name=f"pool{l}") for l in (0, 1)]
        rwbc = [const.tile([P, B_LOC, E], F32, tag=f"rwbc{l}", name=f"rwbc{l}") for l in (0, 1)]

        # ---- x(0) first: chunk 0 issued on the scalar HWDGE ring, chunk 1 on
        # the sync ring, so both transfers stream in parallel lanes from t=0.
        xf_tiles = {}

        def load_x_chunk(b, c, eng):
            xf = xin.tile([P, HW], BF16, tag="xf", name=f"xf{b}{c}")
            eng.dma_start(out=xf, in_=d["x"][b, c * P : (c + 1) * P, :])
            xf_tiles[b, c] = xf

        load_x_chunk(0, 0, nc.scalar)
        load_x_chunk(0, 1, nc.sync)

        # ---- PE warmup: junk matmuls starting as soon as the engines come up
        # keep the HAM activity monitor busy so the PE clock is at 2.4 GHz by
        # the time the first real conv matmuls issue (saves ~3us of K=4/8).
        nc.gpsimd.memset(junk, 0.0)
        warm_ps = cps.tile([P, HW], F32, tag="convps", name="warmps")
        for k in range(N_WARM):
            nc.tensor.matmul(warm_ps[:, 0:512], junk[:, 0:P], junk,
                             start=True, stop=True)

        # ---- ACT warm: force both function-table loads (sigmoid set + copy
        # set) to happen now, during the DMA wait, instead of mid-pipeline.
        warm_t = rsb.tile([P, 2], F32, tag="warm")
        nc.scalar.activation(out=warm_t[:, 0:1], in_=junk[:, 0:1], func=AF.Sigmoid, scale=1.0)
        nc.scalar.activation(out=warm_t[:, 1:2], in_=junk[:, 1:2], func=AF.Copy, scale=1.0)

        # ---- mask constants (vector queue, during DMA wait)
        nc.vector.memset(mask[:, 0], 0.0)
        nc.vector.memset(mask[:, 1], 0.0)
        nc.vector.memset(mask[0:64, 0], 1.0 / 64.0)
        nc.vector.memset(mask[64:128, 1], 1.0 / 64.0)

        # ---- remaining input DMA on the sync HWDGE ring, in consumption
        # order: routing/bn consts, ew1 quarter-granular (ci0-co0 per expert
        # so the first weight combine can start as each expert lands), then
        # interleaved with x(1)..x(3); ew2 last (the sync-queue FIFO keeps
        # its 4.7MB off the startup-critical stream).
        def load_x(b):
            for c in range(CI2):
                load_x_chunk(b, c, nc.sync)

        nc.sync.dma_start(out=rwt_all, in_=d["rwt"])
        nc.sync.dma_start(out=fblob, in_=d["fblob"])
        for e in range(E):
            nc.sync.dma_start(out=ew_sb[0][:, e, 0, 0], in_=d["ew1"][:, e, 0, 0])
        nc.sync.dma_start(out=ew_sb[0][:, :, 1, 0], in_=d["ew1"][:, :, 1, 0])
        nc.sync.dma_start(out=ew_sb[0][:, :, 0, 1], in_=d["ew1"][:, :, 0, 1])
        load_x(1)
        nc.sync.dma_start(out=ew_sb[0][:, :, 1, 1], in_=d["ew1"][:, :, 1, 1])
        load_x(2)
        load_x(3)
        for e in range(E):
            nc.sync.dma_start(out=ew_sb[1][:, e], in_=d["ew2"][:, e])

        # ---- zero the pad borders (vector queue, during DMA wait)
        for b in range(B_LOC):
            v = xpad.rearrange("p b c (r q) -> p b c r q", r=PADW)
            nc.vector.memset(v[:, b, :, 0:PADW:33, :], 0.0)
            nc.vector.memset(v[:, b, :, 1:33, 0:PADW:33], 0.0)
        vo = o1pad.rearrange("p b c (r q) -> p b c r q", r=PADW)
        nc.vector.memset(vo[:, :, :, 0:PADW:33, :], 0.0)
        nc.vector.memset(vo[:, :, :, 1:33, 0:PADW:33], 0.0)

        # ---- copy x (bf16) into padded layout + channel pooling.
        # sample 0 on DVE (bf16 tensor_scalar runs 4x, ~0.33us/chunk, so both
        # pools finish right behind the DMA); samples 1-3 on ACT.
        def cast_x(b):
            for c in range(CI2):
                dst = xpad[:, b, c].rearrange("p (r q) -> p r q", r=PADW)[:, 1:33, 1:33]
                srcv = xf_tiles[b, c].rearrange("p (r q) -> p r q", r=32)
                if b == 0:
                    nc.vector.tensor_scalar(
                        out=dst, in0=srcv, scalar1=1.0, scalar2=0.0,
                        op0=OP.mult, op1=OP.add,
                        accum_out=pool_acc[0][:, b, c : c + 1],
                    )
                else:
                    nc.scalar.activation(
                        out=dst, in_=srcv, func=AF.Copy, scale=1.0,
                        accum_out=pool_acc[0][:, b, c : c + 1],
                    )

        def routing(b, l):
            """pool_acc[l][:, b] -> rwbc[l][:, b] (expert weights broadcast
            to every partition, expert slot order (i, h) = [0, 2, 1, 3])."""
            nc.vector.tensor_copy(pool_bf[l][:, b], pool_acc[l][:, b])
            rt_ps = rps.tile([P, CI2, 1], F32, tag="rpsA", name="rtps")
            for ic in range(2):
                for cc in range(2):
                    nc.tensor.matmul(
                        rt_ps[:, ic],
                        rwt_sb[l][:, cc, ic * P : (ic + 1) * P],
                        pool_bf[l][:, b : b + 1, cc],
                        start=(cc == 0),
                        stop=(cc == 1),
                    )
            rt2 = rsb.tile([P, CI2], BF16, tag="rt2", name="rt2")
            for ic in range(2):
                nc.scalar.activation(
                    out=rt2[:, ic : ic + 1],
                    in_=rt_ps[:, ic],
                    func=AF.Sigmoid,
                    bias=rb_sb[l][:, ic : ic + 1],
                    scale=1.0 / HW,
                )
            # avg over each 64-part expert band + broadcast to all partitions
            rwbc_ps = rps.tile([P, 2, 2], F32, tag="rpsA", name="rwbcps")
            for i in range(2):
                nc.tensor.matmul(rwbc_ps[:, i], mask[:, i], rt2, start=True, stop=True)
            nc.vector.tensor_copy(
                rwbc[l][:, b], rwbc_ps.rearrange("p i h -> p (i h)")
            )

        def pace_mm(rhs):
            """junk matmul that fires as soon as `rhs` is ready — keeps the
            PE's HAM activity monitor from re-throttling during data waits."""
            nc.tensor.matmul(warm_ps[:, 512:1024], junk[:, 0:P], rhs,
                             start=True, stop=True)

        def wcomb_piece(b, l, w, ci, co2=None, e3_act=True, pace=False):
            """combined conv weights for one (ci[,co]) piece: sum_e rw_e*ew_e."""
            if co2 is None:
                wv = w[:, ci].rearrange("p c s f -> p (c s f)")
                srcs = [ew_sb[l][:, e, ci].rearrange("p c f -> p (c f)") for e in range(E)]
                fd = CO2 * QF
            else:
                wv = w[:, ci, co2]
                srcs = [ew_sb[l][:, e, ci, co2] for e in range(E)]
                fd = QF
            for e in range(E):
                sc = rwbc[l][:, b, e : e + 1]
                if e == 0:
                    nc.vector.tensor_scalar(
                        out=wv, in0=srcs[0], scalar1=sc, scalar2=None, op0=OP.mult
                    )
                    continue
                tmp = wcombp.tile([P, fd], BF16, tag="wtmp", name="wtmp")
                if e == 3 and e3_act:
                    nc.scalar.activation(out=tmp, in_=srcs[e], func=AF.Copy, scale=sc)
                else:
                    nc.vector.tensor_scalar(
                        out=tmp, in0=srcs[e], scalar1=sc, scalar2=None, op0=OP.mult
                    )
                if pace:
                    pace_mm(tmp[:, 0:512])
                nc.vector.tensor_add(wv, wv, tmp)

        def new_w():
            return wcombp.tile([P, CI2, CO2, NSH, P], BF16, tag="wcomb", name="wcomb")

        def conv(b, w, srcpad):
            """3x3 same conv: per co-chunk, 18 accumulating matmuls (ci, s)
            x 2 h-halves. Returns two [P, 1024] fp32 psum tiles."""
            psums = []
            for co in range(2):
                ps = cps.tile([P, HW], F32, tag="convps", name="convps")
                for ci in range(2):
                    src34 = srcpad[:, b, ci].rearrange("p (r q) -> p r q", r=PADW)
                    for s in range(NSH):
                        ky, kx = divmod(s, 3)
                        lhsT = w[:, ci, co, s]
                        for hh in range(2):
                            rhs = src34[:, ky + hh * 16 : ky + hh * 16 + 16, kx : kx + 32]
                            nc.tensor.matmul(
                                ps[:, hh * 512 : (hh + 1) * 512],
                                lhsT,
                                rhs,
                                start=(ci == 0 and s == 0),
                                stop=(ci == 1 and s == NSH - 1),
                            )
                psums.append(ps)
            return psums

        def bn1_relu(b, psums):
            for co in range(2):
                dst = o1pad[:, b, co].rearrange("p (r q) -> p r q", r=PADW)[:, 1:33, 1:33]
                nc.scalar.activation(
                    out=dst,
                    in_=psums[co].rearrange("p (r q) -> p r q", r=32),
                    func=AF.Relu,
                    bias=shift_sb[0][:, co : co + 1],
                    scale=inv_sb[0][:, co : co + 1],
                    accum_out=pool_acc[1][:, b, co : co + 1],
                )

        def bn2_res(b, psums):
            for co in range(2):
                res = resp.tile([P, HW], F32, tag="res", name="res")
                resv = res.rearrange("p (r q) -> p r q", r=32)
                xv = xpad[:, b, co].rearrange("p (r q) -> p r q", r=PADW)[:, 1:33, 1:33]
                psv = psums[co].rearrange("p (r q) -> p r q", r=32)
                # res = psum*inv2 + x ; res = max(res + shift2, 0)
                nc.vector.scalar_tensor_tensor(
                    out=resv, in0=psv, scalar=inv_sb[1][:, co : co + 1], in1=xv,
                    op0=OP.mult, op1=OP.add,
                )
                nc.scalar.activation(
                    out=res, in_=res, func=AF.Relu,
                    bias=shift_sb[1][:, co : co + 1], scale=1.0,
                )
                nc.sync.dma_start(out=d["out"][b, co * P : (co + 1) * P, :], in_=res)

        def conv_last(b, w):
            """last conv: independent row-band accumulation groups, each in
            its own psum tile, with the bn2+residual+store epilogue emitted
            right after its group so it overlaps the remaining matmuls
            instead of serializing after the final one. The final band is
            halved so the post-last-matmul tail is ~2x shorter."""
            groups = [(0, 0, 16), (0, 16, 16), (1, 0, 16), (1, 16, 8),
                      (1, 24, 4), (1, 28, 4)]
            for co, r0, nr in groups:
                npix = nr * 32
                ps = cps.tile([P, 512], F32, tag="lastps", name=f"lps{co}{r0}")
                for ci in range(2):
                    src34 = o1pad[:, b, ci].rearrange("p (r q) -> p r q", r=PADW)
                    for s in range(NSH):
                        ky, kx = divmod(s, 3)
                        rhs = src34[:, ky + r0 : ky + r0 + nr, kx : kx + 32]
                        nc.tensor.matmul(
                            ps[:, 0:npix], w[:, ci, co, s], rhs,
                            start=(ci == 0 and s == 0),
                            stop=(ci == 1 and s == NSH - 1),
                        )
                res = resp.tile([P, 512], F32, tag="res512", bufs=2, name=f"res512_{co}{r0}")
                resv = res[:, 0:npix].rearrange("p (r q) -> p r q", r=nr)
                xv = xpad[:, b, co].rearrange("p (r q) -> p r q", r=PADW)[
                    :, 1 + r0 : 1 + r0 + nr, 1:33]
                psv = ps[:, 0:npix].rearrange("p (r q) -> p r q", r=nr)
                nc.vector.scalar_tensor_tensor(
                    out=resv, in0=psv, scalar=inv_sb[1][:, co : co + 1], in1=xv,
                    op0=OP.mult, op1=OP.add,
                )
                nc.scalar.activation(
                    out=res[:, 0:npix], in_=res[:, 0:npix], func=AF.Relu,
                    bias=shift_sb[1][:, co : co + 1], scale=1.0,
                )
                nc.sync.dma_start(
                    out=d["out"][b, co * P : (co + 1) * P, r0 * 32 : r0 * 32 + npix],
                    in_=res[:, 0:npix],
                )

        # ---- main pipeline
        # layer-1 combines are quarter-granular and emitted in DMA-arrival
        # order: engine queues are strict FIFO, so an op gated on late data
        # must not be emitted ahead of ops whose data lands earlier.
        cast_x(0)
        routing(0, 0)
        # static keepalives bridge warmup -> routing; per-expert pacer
        # matmuls fire as each ew1 quarter lands (HAM stays at K=8/8)
        for k in range(4):
            nc.tensor.matmul(warm_ps[:, 512:1024], junk[:, 0:P], junk,
                             start=True, stop=True)
        for e in range(E):
            pace_mm(ew_sb[0][:, e, 0, 0, 0:512])
            pace_mm(ew_sb[0][:, e, 0, 0, 512:1024])
        w1 = {b: new_w() for b in range(B_LOC)}
        wcomb_piece(0, 0, w1[0], ci=0, co2=0, pace=True)
        wcomb_piece(0, 0, w1[0], ci=1, co2=0)
        wcomb_piece(0, 0, w1[0], ci=0, co2=1)
        cast_x(1)
        routing(1, 0)
        wcomb_piece(1, 0, w1[1], ci=0, co2=0)
        wcomb_piece(0, 0, w1[0], ci=1, co2=1)
        wcomb_piece(1, 0, w1[1], ci=1, co2=0)
        wcomb_piece(1, 0, w1[1], ci=0, co2=1)
        wcomb_piece(1, 0, w1[1], ci=1, co2=1)
        for b in range(2, B_LOC):
            cast_x(b)
            routing(b, 0)
            for ci, co2 in ((0, 0), (1, 0), (0, 1), (1, 1)):
                wcomb_piece(b, 0, w1[b], ci=ci, co2=co2)

        w2 = {}
        for b in range(B_LOC):
            ps = conv(b, w1[b], xpad)
            bn1_relu(b, ps)
            routing(b, 1)
            w2[b] = new_w()
            wcomb_piece(b, 1, w2[b], ci=0)
            wcomb_piece(b, 1, w2[b], ci=1)
        for b in range(B_LOC - 1):
            ps = conv(b, w2[b], o1pad)
            bn2_res(b, ps)
        conv_last(B_LOC - 1, w2[B_LOC - 1])


_NC_CACHE = {}


def _build_nc():
    if "nc" not in _NC_CACHE:
        import concourse.bacc as bacc

        # Bacc (not raw Bass): its compile() runs split_sync_waits, which
        # legalizes multi-wait instructions for TRN2's 1-wait-per-inst ISA.
        nc = bacc.Bacc("TRN2", target_bir_lowering=False)
        d = _declare_io(nc)
        with tile.TileContext(nc) as tc:
            _emit(tc, d)
        nc.compile()
        _NC_CACHE["nc"] = nc
    return _NC_CACHE["nc"]


# ---------------------------------------------------------------- host prep

_EPERM = (0, 2, 1, 3)  # expert slot order (band i, chunk h)


def _prep_ew(e_w):
    # [4, 589824] -> [ci_in(128), e_slot, ci_chunk, co_chunk, (ky kx co128)]  bf16
    w = np.asarray(e_w, np.float32).reshape(E, C, CI2, P, 3, 3)
    w = w.transpose(3, 0, 2, 4, 5, 1)  # ci_in, e, ci_chunk, ky, kx, co
    w = w.reshape(P, E, CI2, NSH, CO2, P).transpose(0, 1, 2, 4, 3, 5)
    w = w[:, _EPERM]
    return np.ascontiguousarray(w.reshape(P, E, CI2, CO2, QF)).astype(BF16_NP)


def _prep_rwt(rW):
    # [interm, cin] -> transpose -> [cin_in(128), cin_chunk, interm]
    t = np.asarray(rW, np.float32).T.reshape(CI2, P, C).transpose(1, 0, 2)
    return np.ascontiguousarray(t).astype(BF16_NP)


def _prep_vec(v):
    return np.ascontiguousarray(np.asarray(v, np.float32).reshape(CI2, P).T)


def _fold_bn(g, b, m, v):
    inv = np.asarray(g, np.float32) / np.sqrt(np.asarray(v, np.float32) + EPS)
    shift = np.asarray(b, np.float32) - np.asarray(m, np.float32) * inv
    return _prep_vec(inv), _prep_vec(shift)


def _prep_inputs(inputs):
    inv1, shift1 = _fold_bn(inputs["bn1_gamma"], inputs["bn1_beta"],
                            inputs["bn1_mean"], inputs["bn1_var"])
    inv2, shift2 = _fold_bn(inputs["bn2_gamma"], inputs["bn2_beta"],
                            inputs["bn2_mean"], inputs["bn2_var"])
    fblob = np.concatenate(
        [_prep_vec(inputs["r1_b"]), _prep_vec(inputs["r2_b"]),
         inv1, shift1, inv2, shift2], axis=1
    )
    rwt = np.stack([_prep_rwt(inputs["r1_W"]), _prep_rwt(inputs["r2_W"])], axis=1)
    shared = {
        "ew1": _prep_ew(inputs["e1_w"]),
        "ew2": _prep_ew(inputs["e2_w"]),
        "rwt": np.ascontiguousarray(rwt),
        "fblob": np.ascontiguousarray(fblob),
    }
    x8 = np.ascontiguousarray(
        np.asarray(inputs["x"], np.float32).reshape(N_CORES, B_LOC, C, HW)
    ).astype(BF16_NP)
    return shared, x8


def _run(inputs, trace=False):
    from concourse.bass_utils import run_bass_kernel_spmd

    nc = _build_nc()
    shared, x8 = _prep_inputs(inputs)
    in_maps = [{"x": x8[c], **shared} for c in range(N_CORES)]
    r = run_bass_kernel_spmd(nc, in_maps, list(range(N_CORES)), trace=trace)
    out = np.stack([np.asarray(r.results[c]["out"]) for c in range(N_CORES)])
    return out.reshape(32, C, 32, 32).astype(np.float32), r


def kernel(**inputs):
    out, _ = _run(inputs, trace=False)
    return out


def _install_ntff_shim():
    """The image's antenv package lacks axon_hooks; recreate it and register
    the ctypes NTFF profile hook the way trn_boot would have."""
    import sys
    import types

    if "antenv.axon_hooks" in sys.modules:
        return
    mod = types.ModuleType("antenv.axon_hooks")
    state = {"hook": None}
    mod.set_axon_ntff_profile_hook = lambda h: state.update(hook=h)
    mod.get_axon_ntff_profile_hook = lambda: state["hook"]
    sys.modules["antenv.axon_hooks"] = mod
    import antenv

    antenv.axon_hooks = mod
    try:
        from trn_agent_boot.trn_boot import _ntff_profile_via_ctypes

        mod.set_axon_ntff_profile_hook(
            _ntff_profile_via_ctypes("/opt/axon/libaxon_pjrt.so")
        )
    except Exception as e:  # degrade to no tracing
        print(f"ntff shim failed: {e}")


def run_traced(inputs):
    _install_ntff_shim()
    out, r = _run(inputs, trace=True)
    return out, r


def run_sim(inputs):
    """CoreSim of core 0's shard. Returns [B_LOC, C, 32, 32]."""
    from concourse.bass_interp import CoreSim

    nc = _build_nc()
    shared, x8 = _prep_inputs(inputs)
    sim = CoreSim(nc)
    for k, v in {"x": x8[0], **shared}.items():
        sim.tensor(k)[:] = v
    sim.simulate()
    return np.asarray(sim.tensor("out")).reshape(B_LOC, C, 32, 32).copy()
